# revision 10
# baseline (speedup 1.0000x reference)
"""Distributed Trainium2 Bass kernel for nn_Attention_14955076125142.

Math (reference):
    k_enc = relu(query @ W0.T + b0)
    q_enc = relu(key  @ W1.T + b1)
    energies = rowsum(k_enc * (q_enc @ Wa.T + ba))      # (N,)
    alpha = softmax(energies)                           # (1, N)
    out = alpha @ value                                 # (1, F)

Strategy:
    Shard N=65536 rows across 8 NeuronCores (8192 rows each); replicate
    weights.  Each core computes its shard's energies, the local softmax
    partials (m_i = local max, s_i = sum exp(e - m_i)) and the local
    exp-weighted value sum c_i = sum exp(e_i - m_i) * value_i (1, F).
    The 8 tiny partials ((F+2) floats per core) are combined on the host
    exactly:  M = max m_i;  out = sum_i e^{m_i-M} c_i / sum_i e^{m_i-M} s_i.

    On-device layout: activations live TRANSPOSED ([feature, row]) so the
    three big matmuls chain without on-device transposes; query/key (and
    W0/W1/Wa) are pre-transposed on the host during sharding.  Matmuls run
    as float32r (full-rate PE path on fp32 data).  Energies are produced
    as [128, ntile] (row-on-partition) via ones-matmuls so the phase-2
    exp-weight columns need no transposes.
"""

import numpy as np

N_GLOBAL = 65536
F = 1024
N_CORES = 8
N_LOC = N_GLOBAL // N_CORES  # 8192
P = 128
RB = 512                     # rows per block
KC = F // P                  # contraction chunks (8)
JC = F // P                  # out-feature chunks (8)


def _build(nloc=N_LOC, rb=RB):
    import concourse.bacc as bacc
    import concourse.tile as tile
    import concourse.mybir as mybir
    import concourse.bass_isa as bass_isa

    dt = mybir.dt
    f32 = dt.float32
    mdt = dt.float32r   # matmul-operand dtype
    AF = mybir.ActivationFunctionType
    AX = mybir.AxisListType
    nb = nloc // rb
    ntile = nloc // P
    tpb = rb // P  # row tiles per block (4)

    nc = bacc.Bacc("TRN2", target_bir_lowering=False, debug=False,
                   num_devices=N_CORES)

    qt = nc.dram_tensor("qt", [F, nloc], mdt, kind="ExternalInput")
    kt = nc.dram_tensor("kt", [F, nloc], mdt, kind="ExternalInput")
    v = nc.dram_tensor("v", [nloc, F], mdt, kind="ExternalInput")
    w0t = nc.dram_tensor("w0t", [F, F], mdt, kind="ExternalInput")
    w1t = nc.dram_tensor("w1t", [F, F], mdt, kind="ExternalInput")
    wat = nc.dram_tensor("wat", [F, F], mdt, kind="ExternalInput")
    b0 = nc.dram_tensor("b0", [F], f32, kind="ExternalInput")
    b1 = nc.dram_tensor("b1", [F], f32, kind="ExternalInput")
    ba = nc.dram_tensor("ba", [F], f32, kind="ExternalInput")
    out = nc.dram_tensor("out", [2, F], f32, kind="ExternalOutput")

    with tile.TileContext(nc) as tc:
        with (
            tc.tile_pool(name="wpool", bufs=1) as wpool,
            tc.tile_pool(name="cpool", bufs=1) as cpool,
            tc.tile_pool(name="big", bufs=5) as big,
            tc.tile_pool(name="prod", bufs=2) as prodp,
            tc.tile_pool(name="vpool", bufs=2) as vpool,
            tc.tile_pool(name="ps", bufs=2, space="PSUM") as psp,
            tc.tile_pool(name="psE", bufs=2, space="PSUM") as psEp,
            tc.tile_pool(name="psC", bufs=1, space="PSUM") as psCp,
        ):
            # ---- constants / weights ----
            w0_sb = wpool.tile([P, KC, F], mdt, tag="w0")
            w1_sb = wpool.tile([P, KC, F], mdt, tag="w1")
            wa_sb = wpool.tile([P, KC, F], mdt, tag="wa")
            nc.sync.dma_start(w0_sb[:], w0t.ap().rearrange("(c p) j -> p c j", p=P))
            nc.sync.dma_start(w1_sb[:], w1t.ap().rearrange("(c p) j -> p c j", p=P))
            nc.sync.dma_start(wa_sb[:], wat.ap().rearrange("(c p) j -> p c j", p=P))

            b0_sb = cpool.tile([P, JC], f32, tag="b0")
            b1_sb = cpool.tile([P, JC], f32, tag="b1")
            ba_sb = cpool.tile([P, JC], f32, tag="ba")
            nc.gpsimd.dma_start(b0_sb[:], b0.ap().rearrange("(c p) -> p c", p=P))
            nc.gpsimd.dma_start(b1_sb[:], b1.ap().rearrange("(c p) -> p c", p=P))
            nc.gpsimd.dma_start(ba_sb[:], ba.ap().rearrange("(c p) -> p c", p=P))

            ones_sb = cpool.tile([P, 1], f32, tag="ones")
            nc.gpsimd.memset(ones_sb[:], 1.0)

            E_sb = cpool.tile([P, ntile], f32, tag="E")
            wexp_sb = cpool.tile([P, ntile], mdt, tag="wexp")

            # ---- phase 1: energies per row block ----
            def layer(dst, src, w_sb, b_sb, func):
                for jc in range(JC):
                    ps = psp.tile([P, rb], f32, tag="ps")
                    for kc in range(KC):
                        nc.tensor.matmul(
                            ps[:],
                            w_sb[:, kc, jc * P:(jc + 1) * P],
                            src[:, kc, :],
                            start=(kc == 0),
                            stop=(kc == KC - 1),
                        )
                    nc.scalar.activation(dst[:, jc, :], ps[:], func,
                                         bias=b_sb[:, jc:jc + 1])

            for b in range(nb):
                bs = b * rb
                qt_t = big.tile([P, KC, rb], mdt, tag="acts")
                kt_t = big.tile([P, KC, rb], mdt, tag="acts")
                kenc = big.tile([P, JC, rb], f32, tag="acts")
                qenc = big.tile([P, JC, rb], mdt, tag="acts")
                attn = big.tile([P, JC, rb], f32, tag="acts")
                nc.sync.dma_start(
                    qt_t[:], qt.ap()[:, bs:bs + rb].rearrange("(c p) i -> p c i", p=P))
                nc.sync.dma_start(
                    kt_t[:], kt.ap()[:, bs:bs + rb].rearrange("(c p) i -> p c i", p=P))

                layer(kenc, qt_t, w0_sb, b0_sb, AF.Relu)
                layer(qenc, kt_t, w1_sb, b1_sb, AF.Relu)
                layer(attn, qenc, wa_sb, ba_sb, AF.Identity)

                # energies for the block: E[i] = sum_j kenc[j,i]*attn[j,i]
                for t4 in range(tpb):
                    psE = psEp.tile([P, 1], f32, tag="psE")
                    for jc in range(JC):
                        pr = prodp.tile([P, P], f32, tag="pr")
                        nc.vector.tensor_mul(pr[:],
                                             kenc[:, jc, t4 * P:(t4 + 1) * P],
                                             attn[:, jc, t4 * P:(t4 + 1) * P])
                        nc.tensor.matmul(
                            psE[:],
                            pr[:],
                            ones_sb[:],
                            start=(jc == 0), stop=(jc == JC - 1),
                        )
                    nc.vector.tensor_copy(
                        E_sb[:, b * tpb + t4:b * tpb + t4 + 1], psE[:])

            # ---- local softmax stats ----
            rowmax = cpool.tile([P, 1], f32, tag="rowmax")
            nc.vector.reduce_max(rowmax[:], E_sb[:], axis=AX.X)
            gmax = cpool.tile([P, 1], f32, tag="gmax")
            nc.gpsimd.partition_all_reduce(gmax[:], rowmax[:], channels=P,
                                           reduce_op=bass_isa.ReduceOp.max)
            negm_bc = cpool.tile([P, 1], f32, tag="negm_bc")
            nc.vector.tensor_scalar_mul(negm_bc[:], gmax[:], -1.0)
            nc.scalar.activation(wexp_sb[:], E_sb[:], AF.Exp, bias=negm_bc[:])
            srow = cpool.tile([P, 1], f32, tag="srow")
            nc.vector.reduce_sum(srow[:], wexp_sb[:], axis=AX.X)
            sall = cpool.tile([P, 1], f32, tag="sall")
            nc.gpsimd.partition_all_reduce(sall[:], srow[:], channels=P,
                                           reduce_op=bass_isa.ReduceOp.add)

            # ---- phase 2: c = sum_i exp(e_i - m) * value_i ----
            psc0 = psCp.tile([1, 512], f32, tag="psc0")
            psc1 = psCp.tile([1, 512], f32, tag="psc1")
            for t in range(ntile):
                vt = vpool.tile([P, F], mdt, tag="vt")
                nc.sync.dma_start(vt[:], v.ap()[t * P:(t + 1) * P, :])
                nc.tensor.matmul(psc0[:], wexp_sb[:, t:t + 1],
                                 vt[:, 0:512],
                                 start=(t == 0), stop=(t == ntile - 1))
                nc.tensor.matmul(psc1[:], wexp_sb[:, t:t + 1],
                                 vt[:, 512:F],
                                 start=(t == 0), stop=(t == ntile - 1))

            # ---- pack outputs: row0 = c, row1 = [m, s, 0...] ----
            ostage0 = cpool.tile([1, F], f32, tag="ostage0")
            ostage1 = cpool.tile([1, F], f32, tag="ostage1")
            nc.gpsimd.memset(ostage1[:], 0.0)
            nc.vector.tensor_copy(ostage0[:, 0:512], psc0[:])
            nc.vector.tensor_copy(ostage0[:, 512:F], psc1[:])
            nc.vector.tensor_copy(ostage1[:, 0:1], gmax[0:1, 0:1])
            nc.vector.tensor_copy(ostage1[:, 1:2], sall[0:1, 0:1])
            nc.sync.dma_start(out.ap()[0:1, :], ostage0[:])
            nc.sync.dma_start(out.ap()[1:2, :], ostage1[:])

    nc.compile()
    return nc


def _prepare(inputs, nloc=N_LOC):
    """Host-side sharding/layout prep. Returns (nc, in_maps)."""
    key = np.ascontiguousarray(np.asarray(inputs["key"], dtype=np.float32))
    query = np.ascontiguousarray(np.asarray(inputs["query"], dtype=np.float32))
    value = np.ascontiguousarray(np.asarray(inputs["value"], dtype=np.float32))
    w0t = np.ascontiguousarray(np.asarray(inputs["W0"], dtype=np.float32).T)
    w1t = np.ascontiguousarray(np.asarray(inputs["W1"], dtype=np.float32).T)
    wat = np.ascontiguousarray(np.asarray(inputs["Wa"], dtype=np.float32).T)
    b0 = np.ascontiguousarray(np.asarray(inputs["b0"], dtype=np.float32))
    b1 = np.ascontiguousarray(np.asarray(inputs["b1"], dtype=np.float32))
    ba = np.ascontiguousarray(np.asarray(inputs["ba"], dtype=np.float32))

    qT = np.ascontiguousarray(query.T)  # (F, N)
    kT = np.ascontiguousarray(key.T)

    in_maps = []
    for c in range(N_CORES):
        sl = slice(c * nloc, (c + 1) * nloc)
        in_maps.append({
            "qt": np.ascontiguousarray(qT[:, sl]),
            "kt": np.ascontiguousarray(kT[:, sl]),
            "v": np.ascontiguousarray(value[sl]),
            "w0t": w0t, "w1t": w1t, "wat": wat,
            "b0": b0, "b1": b1, "ba": ba,
        })
    nc = _build(nloc=nloc)
    return nc, in_maps


def _combine(outs):
    """Combine per-core (2, F) partials into the global (1, F) context."""
    m = np.array([o[1, 0] for o in outs], dtype=np.float64)
    s = np.array([o[1, 1] for o in outs], dtype=np.float64)
    c = np.stack([o[0].astype(np.float64) for o in outs])
    M = m.max()
    scale = np.exp(m - M)
    S = float((s * scale).sum())
    C = (c * scale[:, None]).sum(axis=0)
    return (C / S)[None, :].astype(np.float32)


def kernel(**inputs):
    from concourse import bass_utils
    nc, in_maps = _prepare(inputs)
    res = bass_utils.run_bass_kernel_spmd(
        nc, in_maps, core_ids=list(range(N_CORES)))
    return _combine([r["out"] for r in res.results])


# revision 21
# speedup vs baseline: 1.3304x; 1.3304x over previous
"""Distributed Trainium2 Bass kernel for nn_Attention_14955076125142.

Math (reference):
    k_enc = relu(query @ W0.T + b0)
    q_enc = relu(key  @ W1.T + b1)
    energies = rowsum(k_enc * (q_enc @ Wa.T + ba))      # (N,)
    alpha = softmax(energies)                           # (1, N)
    out = alpha @ value                                 # (1, F)

Strategy:
    Shard N=65536 rows across 8 NeuronCores (8192 rows each); replicate
    weights.  Each core computes its shard's softmax partials (running
    per-partition max m_p, sum-exp s_p, and exp-weighted value rows c_p)
    with a flash-attention-style online update fused into the main loop;
    a final on-device reduction collapses partitions, and the 8 tiny
    per-core partials are combined exactly on the host.

    Layouts: L2 (q_enc) runs "transposed" ([feature, row]) off the
    host-pre-transposed key; L1/L3 run "natural" ([row, feature]) with
    host-pre-transposed query / q_encT as the stationary operand, so the
    energies rowsum is a single DVE tensor_tensor_reduce reading the L3
    PSUM directly.  All matmuls are float32r (full-rate PE on fp32 data).
"""

import numpy as np

N_GLOBAL = 65536
F = 1024
N_CORES = 8
N_LOC = N_GLOBAL // N_CORES  # 8192
P = 128
RB = 512                     # rows per block
KC = F // P                  # contraction chunks (8)
JC = F // P                  # out-feature chunks (8)
NEG_BIG = -1.0e30


def _build(nloc=N_LOC, rb=RB, has_bias=False):
    import concourse.bacc as bacc
    import concourse.tile as tile
    import concourse.mybir as mybir
    import concourse.bass_isa as bass_isa

    dt = mybir.dt
    f32 = dt.float32
    mdt = dt.float32r   # matmul-operand dtype
    AF = mybir.ActivationFunctionType
    AX = mybir.AxisListType
    OP = mybir.AluOpType
    nb = nloc // rb
    tpb = rb // P  # row tiles per block (4)

    nc = bacc.Bacc("TRN2", target_bir_lowering=False, debug=False,
                   num_devices=N_CORES)

    qt = nc.dram_tensor("qt", [F, nloc], mdt, kind="ExternalInput")
    kt = nc.dram_tensor("kt", [F, nloc], mdt, kind="ExternalInput")
    v = nc.dram_tensor("v", [nloc, F], f32, kind="ExternalInput")
    w0t = nc.dram_tensor("w0t", [F, F], mdt, kind="ExternalInput")
    w1t = nc.dram_tensor("w1t", [F, F], mdt, kind="ExternalInput")
    wat = nc.dram_tensor("wat", [F, F], mdt, kind="ExternalInput")
    b0 = nc.dram_tensor("b0", [F], mdt, kind="ExternalInput")
    b1 = nc.dram_tensor("b1", [F], f32, kind="ExternalInput")
    ba = nc.dram_tensor("ba", [F], mdt, kind="ExternalInput")
    ones_d = nc.dram_tensor("ones1", [1, P], mdt, kind="ExternalInput")
    out = nc.dram_tensor("out", [2, F], f32, kind="ExternalOutput")

    with tile.TileContext(nc) as tc:
        with (
            tc.tile_pool(name="wpool", bufs=1) as wpool,
            tc.tile_pool(name="cpool", bufs=1) as cpool,
            tc.tile_pool(name="ktp", bufs=2) as ktp,
            tc.tile_pool(name="qep", bufs=1) as qep,
            tc.tile_pool(name="qt4p", bufs=3) as qt4p,
            tc.tile_pool(name="kencp", bufs=2) as kencp,
            tc.tile_pool(name="vtp", bufs=3) as vtp,
            tc.tile_pool(name="smol", bufs=2) as smol,
            tc.tile_pool(name="scrp", bufs=1) as scrp,
            tc.tile_pool(name="ps", bufs=4, space="PSUM") as psp,
            tc.tile_pool(name="psC", bufs=1, space="PSUM") as psCp,
        ):
            # ---- weights / constants ----
            w0_sb = wpool.tile([P, KC, F], mdt, tag="w0")
            w1_sb = wpool.tile([P, KC, F], mdt, tag="w1")
            wa_sb = wpool.tile([P, KC, F], mdt, tag="wa")
            # order matters: first compute is L2 (needs kt + w1)
            for jc in range(JC):
                nc.sync.dma_start(
                    w1_sb[:, :, jc * P:(jc + 1) * P],
                    w1t.ap()[:, jc * P:(jc + 1) * P]
                        .rearrange("(c p) j -> p c j", p=P))
            for jh in range(2):
                nc.sync.dma_start(
                    w0_sb[:, :, jh * 512:(jh + 1) * 512],
                    w0t.ap()[:, jh * 512:(jh + 1) * 512]
                        .rearrange("(c p) j -> p c j", p=P))
            for jh in range(2):
                nc.sync.dma_start(
                    wa_sb[:, :, jh * 512:(jh + 1) * 512],
                    wat.ap()[:, jh * 512:(jh + 1) * 512]
                        .rearrange("(c p) j -> p c j", p=P))

            b1_sb = cpool.tile([P, JC], f32, tag="b1")
            nc.gpsimd.dma_start(b1_sb[:], b1.ap().rearrange("(c p) -> p c", p=P))
            if has_bias:
                onesr = cpool.tile([1, P], mdt, tag="onesr")
                nc.gpsimd.dma_start(onesr[:], ones_d.ap())
                b0_sb = cpool.tile([1, F], mdt, tag="b0r")
                ba_sb = cpool.tile([1, F], mdt, tag="bar")
                nc.gpsimd.dma_start(b0_sb[:], b0.ap())
                nc.gpsimd.dma_start(ba_sb[:], ba.ap())

            ones_f = cpool.tile([P, 1], f32, tag="ones_f")
            nc.gpsimd.memset(ones_f[:], 1.0)

            # online-softmax state (ping-pong pairs; no in-place DVE ops)
            m_ab = [cpool.tile([P, 1], f32, tag="m_a", name="m_a"),
                    cpool.tile([P, 1], f32, tag="m_b", name="m_b")]
            s_ab = [cpool.tile([P, 1], f32, tag="s_a", name="s_a"),
                    cpool.tile([P, 1], f32, tag="s_b", name="s_b")]
            c_ab = [cpool.tile([P, F], f32, tag="c_a", name="c_a"),
                    cpool.tile([P, F], f32, tag="c_b", name="c_b")]
            nc.gpsimd.memset(m_ab[0][:], NEG_BIG)
            nc.gpsimd.memset(s_ab[0][:], 0.0)
            nc.gpsimd.memset(c_ab[0][:], 0.0)

            for b in range(nb):
                bs = b * rb
                kt_t = ktp.tile([P, KC, rb], mdt, tag="kt")
                qenc = qep.tile([P, KC, rb], mdt, tag="qe")
                nc.sync.dma_start(
                    kt_t[:], kt.ap()[:, bs:bs + rb].rearrange("(c p) i -> p c i", p=P))

                # ---- L2 transposed: qencT = relu(W1T.T @ ktT + b1) ----
                for jc in range(JC):
                    ps = psp.tile([P, rb], f32, tag="ps")
                    for kc in range(KC):
                        nc.tensor.matmul(
                            ps[:],
                            w1_sb[:, kc, jc * P:(jc + 1) * P],
                            kt_t[:, kc, :],
                            start=(kc == 0), stop=(kc == KC - 1),
                        )
                    nc.scalar.activation(qenc[:, jc, :], ps[:], AF.Relu,
                                         bias=b1_sb[:, jc:jc + 1])

                for t4 in range(tpb):
                    t_glob = b * tpb + t4
                    off = bs + t4 * P
                    qt_4 = qt4p.tile([P, KC, P], mdt, tag="qt4")
                    nc.sync.dma_start(
                        qt_4[:],
                        qt.ap()[:, off:off + P].rearrange("(c p) i -> p c i", p=P))
                    vt = vtp.tile([P, F], f32, tag="vt")
                    nc.sync.dma_start(vt[:], v.ap()[off:off + P, :])

                    # ---- L1 natural: kenc = relu(q @ W0.T [+ b0])
                    kenc = kencp.tile([P, F], f32, tag="kenc")
                    for jh in range(2):
                        ps1 = psp.tile([P, 512], f32, tag="ps")
                        for kc in range(KC):
                            nc.tensor.matmul(
                                ps1[:],
                                qt_4[:, kc, :],
                                w0_sb[:, kc, jh * 512:(jh + 1) * 512],
                                start=(kc == 0),
                                stop=(kc == KC - 1 and not has_bias),
                            )
                        if has_bias:
                            nc.tensor.matmul(ps1[:], onesr[:],
                                             b0_sb[:, jh * 512:(jh + 1) * 512],
                                             start=False, stop=True)
                        nc.scalar.activation(
                            kenc[:, jh * 512:(jh + 1) * 512], ps1[:], AF.Relu)

                    # ---- L3 natural: attn psum = q_enc @ Wa.T; fused energies
                    e_tmp = smol.tile([P, 1], f32, tag="e_tmp")
                    e_tmp2 = smol.tile([P, 1], f32, tag="e_tmp2")
                    ecol = smol.tile([P, 1], f32, tag="ecol")
                    for jh in range(2):
                        ps3 = psp.tile([P, 512], f32, tag="ps")
                        for kc in range(KC):
                            nc.tensor.matmul(
                                ps3[:],
                                qenc[:, kc, t4 * P:(t4 + 1) * P],
                                wa_sb[:, kc, jh * 512:(jh + 1) * 512],
                                start=(kc == 0),
                                stop=(kc == KC - 1 and not has_bias),
                            )
                        if has_bias:
                            nc.tensor.matmul(ps3[:], onesr[:],
                                             ba_sb[:, jh * 512:(jh + 1) * 512],
                                             start=False, stop=True)
                        # energies partial: rowsum(kenc * attn) over this half
                        pscr = scrp.tile([P, 512], f32, tag="pscr")
                        nc.vector.scalar_tensor_tensor(
                            out=pscr[:],
                            in0=kenc[:, jh * 512:(jh + 1) * 512],
                            scalar=1.0,
                            in1=ps3[:],
                            op0=OP.mult, op1=OP.mult,
                            accum_out=(e_tmp[:] if jh == 0 else e_tmp2[:]),
                        )
                    nc.vector.tensor_add(ecol[:], e_tmp[:], e_tmp2[:])

                    # ---- per-tile online softmax update ----
                    m_old = m_ab[t_glob % 2]
                    m_new = m_ab[(t_glob + 1) % 2]
                    s_old = s_ab[t_glob % 2]
                    s_new = s_ab[(t_glob + 1) % 2]
                    c_old = c_ab[t_glob % 2]
                    c_new = c_ab[(t_glob + 1) % 2]
                    nc.vector.tensor_max(m_new[:], m_old[:], ecol[:])
                    dm = smol.tile([P, 1], f32, tag="dm")
                    nc.vector.tensor_sub(dm[:], m_old[:], m_new[:])
                    sc = smol.tile([P, 1], f32, tag="sc")
                    nc.scalar.activation(sc[:], dm[:], AF.Exp)
                    negm = smol.tile([P, 1], f32, tag="negm")
                    nc.vector.tensor_scalar_mul(negm[:], m_new[:], -1.0)
                    wv = smol.tile([P, 1], f32, tag="wv")
                    nc.scalar.activation(wv[:], ecol[:], AF.Exp, bias=negm[:])
                    nc.vector.scalar_tensor_tensor(
                        out=s_new[:], in0=s_old[:], scalar=sc[:], in1=wv[:],
                        op0=OP.mult, op1=OP.add)
                    ctmp = smol.tile([P, F], f32, tag="ctmp", bufs=1,
                                     name=f"ctmp_{b}_{t4}")
                    nc.vector.tensor_scalar_mul(ctmp[:], c_old[:], sc[:])
                    nc.vector.scalar_tensor_tensor(
                        out=c_new[:], in0=vt[:], scalar=wv[:], in1=ctmp[:],
                        op0=OP.mult, op1=OP.add)

            # ---- final cross-partition reduction ----
            fin = (nb * tpb) % 2
            m_fin = m_ab[fin]
            s_fin = s_ab[fin]
            c_fin = c_ab[fin]
            mall = cpool.tile([P, 1], f32, tag="mall")
            nc.gpsimd.partition_all_reduce(mall[:], m_fin[:], channels=P,
                                           reduce_op=bass_isa.ReduceOp.max)
            df = cpool.tile([P, 1], f32, tag="df")
            nc.vector.tensor_sub(df[:], m_fin[:], mall[:])
            fsc = cpool.tile([P, 1], f32, tag="fsc")
            nc.scalar.activation(fsc[:], df[:], AF.Exp)
            sf = cpool.tile([P, 1], f32, tag="sf")
            nc.vector.tensor_mul(sf[:], s_fin[:], fsc[:])
            sall = cpool.tile([P, 1], f32, tag="sall")
            nc.gpsimd.partition_all_reduce(sall[:], sf[:], channels=P,
                                           reduce_op=bass_isa.ReduceOp.add)
            cf = c_ab[(fin + 1) % 2]
            nc.vector.tensor_scalar_mul(cf[:], c_fin[:], fsc[:])
            psc0 = psCp.tile([1, 512], f32, tag="psc0")
            psc1 = psCp.tile([1, 512], f32, tag="psc1")
            nc.tensor.matmul(psc0[:], ones_f[:], cf[:, 0:512],
                             start=True, stop=True)
            nc.tensor.matmul(psc1[:], ones_f[:], cf[:, 512:F],
                             start=True, stop=True)

            ostage0 = cpool.tile([1, F], f32, tag="ostage0")
            ostage1 = cpool.tile([1, F], f32, tag="ostage1")
            nc.gpsimd.memset(ostage1[:], 0.0)
            nc.vector.tensor_copy(ostage0[:, 0:512], psc0[:])
            nc.vector.tensor_copy(ostage0[:, 512:F], psc1[:])
            nc.vector.tensor_copy(ostage1[:, 0:1], mall[0:1, 0:1])
            nc.vector.tensor_copy(ostage1[:, 1:2], sall[0:1, 0:1])
            nc.sync.dma_start(out.ap()[0:1, :], ostage0[:])
            nc.sync.dma_start(out.ap()[1:2, :], ostage1[:])

    nc.compile()
    return nc


def _prepare(inputs, nloc=N_LOC):
    """Host-side sharding/layout prep. Returns (nc, in_maps)."""
    key = np.ascontiguousarray(np.asarray(inputs["key"], dtype=np.float32))
    query = np.ascontiguousarray(np.asarray(inputs["query"], dtype=np.float32))
    value = np.ascontiguousarray(np.asarray(inputs["value"], dtype=np.float32))
    w0t = np.ascontiguousarray(np.asarray(inputs["W0"], dtype=np.float32).T)
    w1t = np.ascontiguousarray(np.asarray(inputs["W1"], dtype=np.float32).T)
    wat = np.ascontiguousarray(np.asarray(inputs["Wa"], dtype=np.float32).T)
    b0 = np.ascontiguousarray(np.asarray(inputs["b0"], dtype=np.float32))
    b1 = np.ascontiguousarray(np.asarray(inputs["b1"], dtype=np.float32))
    ba = np.ascontiguousarray(np.asarray(inputs["ba"], dtype=np.float32))

    has_bias = bool(np.any(b0 != 0) or np.any(ba != 0))

    qT = np.ascontiguousarray(query.T)  # (F, N)
    kT = np.ascontiguousarray(key.T)

    in_maps = []
    for c in range(N_CORES):
        sl = slice(c * nloc, (c + 1) * nloc)
        in_maps.append({
            "qt": np.ascontiguousarray(qT[:, sl]),
            "kt": np.ascontiguousarray(kT[:, sl]),
            "v": np.ascontiguousarray(value[sl]),
            "w0t": w0t, "w1t": w1t, "wat": wat,
            "b0": b0, "b1": b1, "ba": ba,
            "ones1": np.ones((1, P), dtype=np.float32),
        })
    nc = _build(nloc=nloc, has_bias=has_bias)
    return nc, in_maps


def _combine(outs):
    """Combine per-core (2, F) partials into the global (1, F) context."""
    m = np.array([o[1, 0] for o in outs], dtype=np.float64)
    s = np.array([o[1, 1] for o in outs], dtype=np.float64)
    c = np.stack([o[0].astype(np.float64) for o in outs])
    M = m.max()
    scale = np.exp(m - M)
    S = float((s * scale).sum())
    C = (c * scale[:, None]).sum(axis=0)
    return (C / S)[None, :].astype(np.float32)


def kernel(**inputs):
    from concourse import bass_utils
    nc, in_maps = _prepare(inputs)
    res = bass_utils.run_bass_kernel_spmd(
        nc, in_maps, core_ids=list(range(N_CORES)))
    return _combine([r["out"] for r in res.results])


# revision 22
# speedup vs baseline: 1.3503x; 1.0149x over previous
"""Distributed Trainium2 Bass kernel for nn_Attention_14955076125142.

Math (reference):
    k_enc = relu(query @ W0.T + b0)
    q_enc = relu(key  @ W1.T + b1)
    energies = rowsum(k_enc * (q_enc @ Wa.T + ba))      # (N,)
    alpha = softmax(energies)                           # (1, N)
    out = alpha @ value                                 # (1, F)

Strategy:
    Shard N=65536 rows across 8 NeuronCores (8192 rows each); replicate
    weights.  Each core computes its shard's softmax partials (running
    per-partition max m_p, sum-exp s_p, and exp-weighted value rows c_p)
    with a flash-attention-style online update fused into the main loop;
    a final on-device reduction collapses partitions, and the 8 tiny
    per-core partials are combined exactly on the host.

    Layouts: L2 (q_enc) runs "transposed" ([feature, row]) off the
    host-pre-transposed key; L1/L3 run "natural" ([row, feature]) with
    host-pre-transposed query / q_encT as the stationary operand, so the
    energies rowsum is a single DVE tensor_tensor_reduce reading the L3
    PSUM directly.  All matmuls are float32r (full-rate PE on fp32 data).
"""

import numpy as np

N_GLOBAL = 65536
F = 1024
N_CORES = 8
N_LOC = N_GLOBAL // N_CORES  # 8192
P = 128
RB = 512                     # rows per block
KC = F // P                  # contraction chunks (8)
JC = F // P                  # out-feature chunks (8)
NEG_BIG = -1.0e30


def _build(nloc=N_LOC, rb=RB, has_bias=False):
    import concourse.bacc as bacc
    import concourse.tile as tile
    import concourse.mybir as mybir
    import concourse.bass_isa as bass_isa

    dt = mybir.dt
    f32 = dt.float32
    mdt = dt.float32r   # matmul-operand dtype
    AF = mybir.ActivationFunctionType
    AX = mybir.AxisListType
    OP = mybir.AluOpType
    nb = nloc // rb
    tpb = rb // P  # row tiles per block (4)

    nc = bacc.Bacc("TRN2", target_bir_lowering=False, debug=False,
                   num_devices=N_CORES)

    qt = nc.dram_tensor("qt", [F, nloc], mdt, kind="ExternalInput")
    kt = nc.dram_tensor("kt", [F, nloc], mdt, kind="ExternalInput")
    v = nc.dram_tensor("v", [nloc, F], f32, kind="ExternalInput")
    w0t = nc.dram_tensor("w0t", [F, F], mdt, kind="ExternalInput")
    w1t = nc.dram_tensor("w1t", [F, F], mdt, kind="ExternalInput")
    wat = nc.dram_tensor("wat", [F, F], mdt, kind="ExternalInput")
    b0 = nc.dram_tensor("b0", [F], mdt, kind="ExternalInput")
    b1 = nc.dram_tensor("b1", [F], f32, kind="ExternalInput")
    ba = nc.dram_tensor("ba", [F], mdt, kind="ExternalInput")
    ones_d = nc.dram_tensor("ones1", [1, P], mdt, kind="ExternalInput")
    out = nc.dram_tensor("out", [2, F], f32, kind="ExternalOutput")

    with tile.TileContext(nc) as tc:
        with (
            tc.tile_pool(name="wpool", bufs=1) as wpool,
            tc.tile_pool(name="cpool", bufs=1) as cpool,
            tc.tile_pool(name="ktp", bufs=2) as ktp,
            tc.tile_pool(name="qep", bufs=1) as qep,
            tc.tile_pool(name="qt4p", bufs=3) as qt4p,
            tc.tile_pool(name="kencp", bufs=2) as kencp,
            tc.tile_pool(name="vtp", bufs=3) as vtp,
            tc.tile_pool(name="smol", bufs=2) as smol,
            tc.tile_pool(name="scrp", bufs=1) as scrp,
            tc.tile_pool(name="ps", bufs=4, space="PSUM") as psp,
            tc.tile_pool(name="psC", bufs=1, space="PSUM") as psCp,
        ):
            # ---- weights / constants ----
            # per-chunk weight tiles so the first matmuls only wait on the
            # chunks they read (whole-tile deps would serialize startup)
            w1_t = [wpool.tile([P, KC, P], mdt, tag=f"w1_{jc}",
                               name=f"w1_{jc}") for jc in range(JC)]
            w0_t = [wpool.tile([P, KC, 512], mdt, tag=f"w0_{jh}",
                               name=f"w0_{jh}") for jh in range(2)]
            wa_t = [wpool.tile([P, KC, 512], mdt, tag=f"wa_{jh}",
                               name=f"wa_{jh}") for jh in range(2)]
            # first compute is L2 (needs kt + w1 chunks); w0/wa issued inside
            # block 0 after its kt DMA so the pipe fills ASAP
            for jc in range(JC):
                nc.sync.dma_start(
                    w1_t[jc][:],
                    w1t.ap()[:, jc * P:(jc + 1) * P]
                        .rearrange("(c p) j -> p c j", p=P))

            b1_sb = cpool.tile([P, JC], f32, tag="b1")
            nc.gpsimd.dma_start(b1_sb[:], b1.ap().rearrange("(c p) -> p c", p=P))
            if has_bias:
                onesr = cpool.tile([1, P], mdt, tag="onesr")
                nc.gpsimd.dma_start(onesr[:], ones_d.ap())
                b0_sb = cpool.tile([1, F], mdt, tag="b0r")
                ba_sb = cpool.tile([1, F], mdt, tag="bar")
                nc.gpsimd.dma_start(b0_sb[:], b0.ap())
                nc.gpsimd.dma_start(ba_sb[:], ba.ap())

            ones_f = cpool.tile([P, 1], f32, tag="ones_f")
            nc.gpsimd.memset(ones_f[:], 1.0)

            # online-softmax state (ping-pong pairs; no in-place DVE ops)
            m_ab = [cpool.tile([P, 1], f32, tag="m_a", name="m_a"),
                    cpool.tile([P, 1], f32, tag="m_b", name="m_b")]
            s_ab = [cpool.tile([P, 1], f32, tag="s_a", name="s_a"),
                    cpool.tile([P, 1], f32, tag="s_b", name="s_b")]
            c_ab = [cpool.tile([P, F], f32, tag="c_a", name="c_a"),
                    cpool.tile([P, F], f32, tag="c_b", name="c_b")]
            nc.gpsimd.memset(m_ab[0][:], NEG_BIG)
            nc.gpsimd.memset(s_ab[0][:], 0.0)
            nc.gpsimd.memset(c_ab[0][:], 0.0)

            for b in range(nb):
                bs = b * rb
                kt_t = ktp.tile([P, KC, rb], mdt, tag="kt")
                qenc = qep.tile([P, KC, rb], mdt, tag="qe")
                nc.sync.dma_start(
                    kt_t[:], kt.ap()[:, bs:bs + rb].rearrange("(c p) i -> p c i", p=P))
                if b == 0:
                    for jh in range(2):
                        nc.sync.dma_start(
                            w0_t[jh][:],
                            w0t.ap()[:, jh * 512:(jh + 1) * 512]
                                .rearrange("(c p) j -> p c j", p=P))
                    for jh in range(2):
                        nc.sync.dma_start(
                            wa_t[jh][:],
                            wat.ap()[:, jh * 512:(jh + 1) * 512]
                                .rearrange("(c p) j -> p c j", p=P))

                # ---- L2 transposed: qencT = relu(W1T.T @ ktT + b1) ----
                for jc in range(JC):
                    ps = psp.tile([P, rb], f32, tag="ps")
                    for kc in range(KC):
                        nc.tensor.matmul(
                            ps[:],
                            w1_t[jc][:, kc, :],
                            kt_t[:, kc, :],
                            start=(kc == 0), stop=(kc == KC - 1),
                        )
                    nc.scalar.activation(qenc[:, jc, :], ps[:], AF.Relu,
                                         bias=b1_sb[:, jc:jc + 1])

                for t4 in range(tpb):
                    t_glob = b * tpb + t4
                    off = bs + t4 * P
                    qt_4 = qt4p.tile([P, KC, P], mdt, tag="qt4")
                    nc.sync.dma_start(
                        qt_4[:],
                        qt.ap()[:, off:off + P].rearrange("(c p) i -> p c i", p=P))
                    vt = vtp.tile([P, F], f32, tag="vt")
                    nc.sync.dma_start(vt[:], v.ap()[off:off + P, :])

                    # ---- L1 natural: kenc = relu(q @ W0.T [+ b0])
                    kenc = kencp.tile([P, F], f32, tag="kenc")
                    for jh in range(2):
                        ps1 = psp.tile([P, 512], f32, tag="ps")
                        for kc in range(KC):
                            nc.tensor.matmul(
                                ps1[:],
                                qt_4[:, kc, :],
                                w0_t[jh][:, kc, :],
                                start=(kc == 0),
                                stop=(kc == KC - 1 and not has_bias),
                            )
                        if has_bias:
                            nc.tensor.matmul(ps1[:], onesr[:],
                                             b0_sb[:, jh * 512:(jh + 1) * 512],
                                             start=False, stop=True)
                        nc.scalar.activation(
                            kenc[:, jh * 512:(jh + 1) * 512], ps1[:], AF.Relu)

                    # ---- L3 natural: attn psum = q_enc @ Wa.T; fused energies
                    e_tmp = smol.tile([P, 1], f32, tag="e_tmp")
                    e_tmp2 = smol.tile([P, 1], f32, tag="e_tmp2")
                    ecol = smol.tile([P, 1], f32, tag="ecol")
                    for jh in range(2):
                        ps3 = psp.tile([P, 512], f32, tag="ps")
                        for kc in range(KC):
                            nc.tensor.matmul(
                                ps3[:],
                                qenc[:, kc, t4 * P:(t4 + 1) * P],
                                wa_t[jh][:, kc, :],
                                start=(kc == 0),
                                stop=(kc == KC - 1 and not has_bias),
                            )
                        if has_bias:
                            nc.tensor.matmul(ps3[:], onesr[:],
                                             ba_sb[:, jh * 512:(jh + 1) * 512],
                                             start=False, stop=True)
                        # energies partial: rowsum(kenc * attn) over this half
                        pscr = scrp.tile([P, 512], f32, tag="pscr")
                        nc.vector.scalar_tensor_tensor(
                            out=pscr[:],
                            in0=kenc[:, jh * 512:(jh + 1) * 512],
                            scalar=1.0,
                            in1=ps3[:],
                            op0=OP.mult, op1=OP.mult,
                            accum_out=(e_tmp[:] if jh == 0 else e_tmp2[:]),
                        )
                    nc.vector.tensor_add(ecol[:], e_tmp[:], e_tmp2[:])

                    # ---- per-tile online softmax update ----
                    m_old = m_ab[t_glob % 2]
                    m_new = m_ab[(t_glob + 1) % 2]
                    s_old = s_ab[t_glob % 2]
                    s_new = s_ab[(t_glob + 1) % 2]
                    c_old = c_ab[t_glob % 2]
                    c_new = c_ab[(t_glob + 1) % 2]
                    nc.vector.tensor_max(m_new[:], m_old[:], ecol[:])
                    dm = smol.tile([P, 1], f32, tag="dm")
                    nc.vector.tensor_sub(dm[:], m_old[:], m_new[:])
                    sc = smol.tile([P, 1], f32, tag="sc")
                    nc.scalar.activation(sc[:], dm[:], AF.Exp)
                    negm = smol.tile([P, 1], f32, tag="negm")
                    nc.vector.tensor_scalar_mul(negm[:], m_new[:], -1.0)
                    wv = smol.tile([P, 1], f32, tag="wv")
                    nc.scalar.activation(wv[:], ecol[:], AF.Exp, bias=negm[:])
                    nc.vector.scalar_tensor_tensor(
                        out=s_new[:], in0=s_old[:], scalar=sc[:], in1=wv[:],
                        op0=OP.mult, op1=OP.add)
                    ctmp = smol.tile([P, F], f32, tag="ctmp", bufs=1,
                                     name=f"ctmp_{b}_{t4}")
                    nc.vector.tensor_scalar_mul(ctmp[:], c_old[:], sc[:])
                    nc.vector.scalar_tensor_tensor(
                        out=c_new[:], in0=vt[:], scalar=wv[:], in1=ctmp[:],
                        op0=OP.mult, op1=OP.add)

            # ---- final cross-partition reduction ----
            fin = (nb * tpb) % 2
            m_fin = m_ab[fin]
            s_fin = s_ab[fin]
            c_fin = c_ab[fin]
            mall = cpool.tile([P, 1], f32, tag="mall")
            nc.gpsimd.partition_all_reduce(mall[:], m_fin[:], channels=P,
                                           reduce_op=bass_isa.ReduceOp.max)
            df = cpool.tile([P, 1], f32, tag="df")
            nc.vector.tensor_sub(df[:], m_fin[:], mall[:])
            fsc = cpool.tile([P, 1], f32, tag="fsc")
            nc.scalar.activation(fsc[:], df[:], AF.Exp)
            sf = cpool.tile([P, 1], f32, tag="sf")
            nc.vector.tensor_mul(sf[:], s_fin[:], fsc[:])
            sall = cpool.tile([P, 1], f32, tag="sall")
            nc.gpsimd.partition_all_reduce(sall[:], sf[:], channels=P,
                                           reduce_op=bass_isa.ReduceOp.add)
            cf = c_ab[(fin + 1) % 2]
            nc.vector.tensor_scalar_mul(cf[:], c_fin[:], fsc[:])
            psc0 = psCp.tile([1, 512], f32, tag="psc0")
            psc1 = psCp.tile([1, 512], f32, tag="psc1")
            nc.tensor.matmul(psc0[:], ones_f[:], cf[:, 0:512],
                             start=True, stop=True)
            nc.tensor.matmul(psc1[:], ones_f[:], cf[:, 512:F],
                             start=True, stop=True)

            ostage0 = cpool.tile([1, F], f32, tag="ostage0")
            ostage1 = cpool.tile([1, F], f32, tag="ostage1")
            nc.gpsimd.memset(ostage1[:], 0.0)
            nc.vector.tensor_copy(ostage0[:, 0:512], psc0[:])
            nc.vector.tensor_copy(ostage0[:, 512:F], psc1[:])
            nc.vector.tensor_copy(ostage1[:, 0:1], mall[0:1, 0:1])
            nc.vector.tensor_copy(ostage1[:, 1:2], sall[0:1, 0:1])
            nc.sync.dma_start(out.ap()[0:1, :], ostage0[:])
            nc.sync.dma_start(out.ap()[1:2, :], ostage1[:])

    nc.compile()
    return nc


def _prepare(inputs, nloc=N_LOC):
    """Host-side sharding/layout prep. Returns (nc, in_maps)."""
    key = np.ascontiguousarray(np.asarray(inputs["key"], dtype=np.float32))
    query = np.ascontiguousarray(np.asarray(inputs["query"], dtype=np.float32))
    value = np.ascontiguousarray(np.asarray(inputs["value"], dtype=np.float32))
    w0t = np.ascontiguousarray(np.asarray(inputs["W0"], dtype=np.float32).T)
    w1t = np.ascontiguousarray(np.asarray(inputs["W1"], dtype=np.float32).T)
    wat = np.ascontiguousarray(np.asarray(inputs["Wa"], dtype=np.float32).T)
    b0 = np.ascontiguousarray(np.asarray(inputs["b0"], dtype=np.float32))
    b1 = np.ascontiguousarray(np.asarray(inputs["b1"], dtype=np.float32))
    ba = np.ascontiguousarray(np.asarray(inputs["ba"], dtype=np.float32))

    has_bias = bool(np.any(b0 != 0) or np.any(ba != 0))

    qT = np.ascontiguousarray(query.T)  # (F, N)
    kT = np.ascontiguousarray(key.T)

    in_maps = []
    for c in range(N_CORES):
        sl = slice(c * nloc, (c + 1) * nloc)
        in_maps.append({
            "qt": np.ascontiguousarray(qT[:, sl]),
            "kt": np.ascontiguousarray(kT[:, sl]),
            "v": np.ascontiguousarray(value[sl]),
            "w0t": w0t, "w1t": w1t, "wat": wat,
            "b0": b0, "b1": b1, "ba": ba,
            "ones1": np.ones((1, P), dtype=np.float32),
        })
    nc = _build(nloc=nloc, has_bias=has_bias)
    return nc, in_maps


def _combine(outs):
    """Combine per-core (2, F) partials into the global (1, F) context."""
    m = np.array([o[1, 0] for o in outs], dtype=np.float64)
    s = np.array([o[1, 1] for o in outs], dtype=np.float64)
    c = np.stack([o[0].astype(np.float64) for o in outs])
    M = m.max()
    scale = np.exp(m - M)
    S = float((s * scale).sum())
    C = (c * scale[:, None]).sum(axis=0)
    return (C / S)[None, :].astype(np.float32)


def kernel(**inputs):
    from concourse import bass_utils
    nc, in_maps = _prepare(inputs)
    res = bass_utils.run_bass_kernel_spmd(
        nc, in_maps, core_ids=list(range(N_CORES)))
    return _combine([r["out"] for r in res.results])
